# revision 4
# baseline (speedup 1.0000x reference)
"""Trainium2 Bass kernel for nn_Gen_64699387347198.

Model (reference):
    mu, var = batch stats of x over N=65536 rows (training-mode BatchNorm1d)
    xn = (x - mu) * rsqrt(var + eps) * gamma + beta
    raw_feats = tanh(xn)
    out = tanh(((xn @ W1.T + b1) @ W2.T + b2) @ Wf.T + bf)

Key algebraic structure: there is no nonlinearity between the three Linear
layers, so they collapse to a single 256x256 matrix Wc = Wf @ W2 @ W1 with
bias bc = bf + Wf @ b2 + (Wf @ W2) @ b1, and the BatchNorm affine folds in:
    out = tanh(x @ (Wc diag(a)).T + (bc + Wc @ b)),  a = gamma*rsig,
    b = beta - mu*a.
The fold is computed ON DEVICE (it only needs the weights, so it overlaps
with the x DMA and the stats all-reduce).

Sharding: pure data parallel over the batch dim across 8 NeuronCores
(8192 rows/core). Activations are kept feature-major ([feature, row]) on
device so BN stats are per-partition free-dim reductions and matmuls need
no activation transposes; the host pre-transposes each x shard and
re-transposes the two outputs.

BatchNorm statistics use an exact cross-core AllReduce of per-shard
(sum x, sum x^2): [128, 4] fp32 through internal-DRAM bounce buffers.
"""

import numpy as np

from concourse import bacc, bass, mybir, tile, bass_utils

F32 = mybir.dt.float32
AF = mybir.ActivationFunctionType
ALU = mybir.AluOpType

N_CORES = 8
N_FULL = 65536
D = 256
H1 = 512
H2 = 2048
NR = N_FULL // N_CORES          # rows per core = 8192
SUB = 2048                      # stats/raw subtile width (free dim)
NSUB = NR // SUB                # 4 subtiles per feature chunk
TW = 512                        # main-loop row-tile width (psum free dim)
NT = NR // TW                   # 16 row tiles
EPS = 1e-5

_CACHE = {}


def _build():
    """Build + compile the 8-core SPMD module once per process."""
    if "nc" in _CACHE:
        return _CACHE["nc"]

    nc = bacc.Bacc("TRN2", target_bir_lowering=False, debug=False,
                   num_devices=N_CORES)

    # ---- DRAM I/O (per core) ----
    xT = nc.dram_tensor("xT", [D, NR], F32, kind="ExternalInput").ap()
    w1 = nc.dram_tensor("w1", [H1, D], F32, kind="ExternalInput").ap()
    w2 = nc.dram_tensor("w2", [H2, H1], F32, kind="ExternalInput").ap()
    wfT = nc.dram_tensor("wfT", [H2, D], F32, kind="ExternalInput").ap()
    b1c = nc.dram_tensor("b1c", [128, H1 // 128], F32, kind="ExternalInput").ap()
    b2c = nc.dram_tensor("b2c", [128, H2 // 128], F32, kind="ExternalInput").ap()
    bfc = nc.dram_tensor("bfc", [128, 2], F32, kind="ExternalInput").ap()
    gamc = nc.dram_tensor("gamc", [128, 2], F32, kind="ExternalInput").ap()
    betc = nc.dram_tensor("betc", [128, 2], F32, kind="ExternalInput").ap()
    rawT = nc.dram_tensor("rawT", [D, NR], F32, kind="ExternalOutput").ap()
    outT = nc.dram_tensor("outT", [D, NR], F32, kind="ExternalOutput").ap()

    with tile.TileContext(nc) as tc:
        with (
            tc.tile_pool(name="xp", bufs=1) as xp,
            tc.tile_pool(name="wp", bufs=1) as wp,
            tc.tile_pool(name="sm", bufs=1) as sm,
            tc.tile_pool(name="wk", bufs=2) as wk,
            tc.tile_pool(name="ob", bufs=4) as ob,
            tc.tile_pool(name="pf", bufs=2, space="PSUM") as pf,
            tc.tile_pool(name="psmall", bufs=2, space="PSUM") as psmall,
            tc.tile_pool(name="pm", bufs=4, space="PSUM") as pm,
            tc.tile_pool(name="dram", bufs=1, space="DRAM") as dram,
        ):
            # ---- persistent SBUF tiles ----
            xs = [xp.tile([128, NR], F32, tag=f"x{jc}", name=f"x{jc}") for jc in range(2)]
            w2_sb = wp.tile([128, 16 * H1], F32, tag="w2")      # chunk k at [:, H1*k:]
            wfT_sb = wp.tile([128, 16 * D], F32, tag="wfT")     # chunk k at [:, D*k:]
            w1_sb = wp.tile([128, 4 * D], F32, tag="w1")        # chunk k at [:, D*k:]
            b1_sb = sm.tile([128, 4], F32, tag="b1")
            b2_sb = sm.tile([128, 16], F32, tag="b2")
            bf_sb = sm.tile([128, 2], F32, tag="bf")
            gam_sb = sm.tile([128, 2], F32, tag="gam")
            bet_sb = sm.tile([128, 2], F32, tag="bet")
            m1t_sb = sm.tile([128, 4 * D], F32, tag="m1t")      # (Wf@W2).T chunks
            wcT_sb = sm.tile([128, 2 * D], F32, tag="wcT")      # Wc.T chunks
            wpT_sb = sm.tile([128, 2 * D], F32, tag="wpT")      # (Wc diag(a)).T
            bc_sb = sm.tile([128, 2], F32, tag="bc")            # bias chain
            bp_sb = sm.tile([128, 2], F32, tag="bp")            # final bias b'
            s1p = sm.tile([128, 2 * NSUB], F32, tag="s1p")
            s2p = sm.tile([128, 2 * NSUB], F32, tag="s2p")
            stats_sb = sm.tile([128, 4], F32, tag="stats")      # s1c0,s1c1,s2c0,s2c1
            gstats_sb = sm.tile([128, 4], F32, tag="gstats")    # after all-reduce
            stw = sm.tile([128, 16], F32, tag="stw")            # stats scratch cols
            a_sb = sm.tile([128, 2], F32, tag="a")
            b_sb = sm.tile([128, 2], F32, tag="b")

            st_in = dram.tile([128, 4], F32)
            st_out = dram.tile([128, 4], F32)

            # ---- input DMAs: x subtiles first (stats critical path) ----
            for jc in range(2):
                for k in range(NSUB):
                    nc.sync.dma_start(
                        xs[jc][:, SUB * k: SUB * (k + 1)],
                        xT[128 * jc: 128 * (jc + 1), SUB * k: SUB * (k + 1)],
                    )
            # weights (single big DMAs, 3D APs: (k p) n -> p k n)
            nc.sync.dma_start(
                w2_sb[:].rearrange("p (k n) -> p k n", k=16),
                w2.rearrange("(k p) n -> p k n", p=128),
            )
            nc.sync.dma_start(
                wfT_sb[:].rearrange("p (k n) -> p k n", k=16),
                wfT.rearrange("(k p) n -> p k n", p=128),
            )
            nc.sync.dma_start(
                w1_sb[:].rearrange("p (k n) -> p k n", k=4),
                w1.rearrange("(k p) n -> p k n", p=128),
            )
            nc.sync.dma_start(b1_sb[:], b1c)
            nc.sync.dma_start(b2_sb[:], b2c)
            nc.sync.dma_start(bf_sb[:], bfc)
            nc.sync.dma_start(gam_sb[:], gamc)
            nc.sync.dma_start(bet_sb[:], betc)

            # ---- stats: per-subtile sum(x) and sum(x^2) on DVE ----
            for jc in range(2):
                for k in range(NSUB):
                    s = NSUB * jc + k
                    xsl = xs[jc][:, SUB * k: SUB * (k + 1)]
                    nc.vector.tensor_reduce(
                        s1p[:, s: s + 1], xsl, axis=mybir.AxisListType.X,
                        op=ALU.add,
                    )
                    sq = wk.tile([128, SUB], F32, tag="sq")
                    # sum(x^2) fused on ACT: Square writes sq, accum_out sums
                    # it along the free dim. (tensor_tensor_reduce hangs on HW
                    # for this op combo, so the square lives on ScalarE.)
                    nc.scalar.activation(
                        sq[:], xsl, AF.Square, accum_out=s2p[:, s: s + 1],
                    )
            for jc in range(2):
                nc.vector.tensor_reduce(
                    stats_sb[:, jc: jc + 1],
                    s1p[:, NSUB * jc: NSUB * (jc + 1)],
                    axis=mybir.AxisListType.X, op=ALU.add,
                )
                nc.vector.tensor_reduce(
                    stats_sb[:, 2 + jc: 3 + jc],
                    s2p[:, NSUB * jc: NSUB * (jc + 1)],
                    axis=mybir.AxisListType.X, op=ALU.add,
                )

            # ---- cross-core all-reduce of the packed stats ----
            nc.sync.dma_start(st_in[:], stats_sb[:])
            nc.gpsimd.collective_compute(
                "AllReduce", ALU.add,
                replica_groups=[list(range(N_CORES))],
                ins=[st_in.opt()], outs=[st_out.opt()],
            )
            nc.sync.dma_start(gstats_sb[:], st_out[:])

            # ---- weight fold (stats-independent; overlaps DMA + AR) ----
            # C1: M1t = (Wf @ W2).T   [H1, D] in 4 chunks of 128 partitions
            for m in range(4):
                ps = pf.tile([128, D], F32, tag="pf")
                for k in range(16):
                    nc.tensor.matmul(
                        ps[:],
                        w2_sb[:, H1 * k + 128 * m: H1 * k + 128 * (m + 1)],
                        wfT_sb[:, D * k: D * (k + 1)],
                        start=(k == 0), stop=(k == 15),
                    )
                nc.vector.tensor_copy(m1t_sb[:, D * m: D * (m + 1)], ps[:])
            # C2: WcT = W1.T-contract: WcT[j,i] = sum_m W1[m,j] M1t[m,i]
            for j in range(2):
                ps = pf.tile([128, D], F32, tag="pf")
                for k in range(4):
                    nc.tensor.matmul(
                        ps[:],
                        w1_sb[:, D * k + 128 * j: D * k + 128 * (j + 1)],
                        m1t_sb[:, D * k: D * (k + 1)],
                        start=(k == 0), stop=(k == 3),
                    )
                nc.vector.tensor_copy(wcT_sb[:, D * j: D * (j + 1)], ps[:])
            # bias chain bc = Wf@b2 + M1@b1 + bf  (two 128-chunks over out dim)
            for i in range(2):
                ps = psmall.tile([128, 1], F32, tag="ps")
                for k in range(16):
                    nc.tensor.matmul(
                        ps[:],
                        wfT_sb[:, D * k + 128 * i: D * k + 128 * (i + 1)],
                        b2_sb[:, k: k + 1],
                        start=(k == 0), stop=False,
                    )
                for k in range(4):
                    nc.tensor.matmul(
                        ps[:],
                        m1t_sb[:, D * k + 128 * i: D * k + 128 * (i + 1)],
                        b1_sb[:, k: k + 1],
                        start=False, stop=(k == 3),
                    )
                nc.vector.tensor_tensor(
                    bc_sb[:, i: i + 1], ps[:], bf_sb[:, i: i + 1], op=ALU.add,
                )

            # ---- stats math: a = gamma*rsig, b = beta - mu*a ----
            inv_n = 1.0 / float(N_FULL)
            for jc in range(2):
                mu = stw[:, 8 * jc + 0: 8 * jc + 1]
                ex2 = stw[:, 8 * jc + 1: 8 * jc + 2]
                msq = stw[:, 8 * jc + 2: 8 * jc + 3]
                var = stw[:, 8 * jc + 3: 8 * jc + 4]
                sd = stw[:, 8 * jc + 4: 8 * jc + 5]
                rsig = stw[:, 8 * jc + 5: 8 * jc + 6]
                mua = stw[:, 8 * jc + 6: 8 * jc + 7]
                nc.vector.tensor_scalar_mul(mu, gstats_sb[:, jc: jc + 1], inv_n)
                nc.vector.tensor_scalar_mul(ex2, gstats_sb[:, 2 + jc: 3 + jc], inv_n)
                nc.vector.tensor_tensor(msq, mu, mu, op=ALU.mult)
                nc.vector.tensor_tensor(var, ex2, msq, op=ALU.subtract)
                nc.vector.tensor_scalar_add(var, var, EPS)
                nc.scalar.activation(sd, var, AF.Sqrt)
                nc.vector.reciprocal(rsig, sd)
                nc.vector.tensor_tensor(
                    a_sb[:, jc: jc + 1], gam_sb[:, jc: jc + 1], rsig, op=ALU.mult)
                nc.vector.tensor_tensor(mua, mu, a_sb[:, jc: jc + 1], op=ALU.mult)
                nc.vector.tensor_tensor(
                    b_sb[:, jc: jc + 1], bet_sb[:, jc: jc + 1], mua,
                    op=ALU.subtract)

            # ---- fold BN affine into the weights: W'T = diag(a) @ WcT ----
            for jc in range(2):
                nc.vector.tensor_scalar_mul(
                    wpT_sb[:, D * jc: D * (jc + 1)],
                    wcT_sb[:, D * jc: D * (jc + 1)],
                    a_sb[:, jc: jc + 1],
                )
            # b' = Wc @ b + bc
            for i in range(2):
                ps = psmall.tile([128, 1], F32, tag="ps")
                for jc in range(2):
                    nc.tensor.matmul(
                        ps[:],
                        wcT_sb[:, D * jc + 128 * i: D * jc + 128 * (i + 1)],
                        b_sb[:, jc: jc + 1],
                        start=(jc == 0), stop=(jc == 1),
                    )
                nc.vector.tensor_tensor(
                    bp_sb[:, i: i + 1], ps[:], bc_sb[:, i: i + 1], op=ALU.add,
                )

            # ---- raw_feats: tanh(a*x + b) on ACT, streamed out ----
            for jc in range(2):
                for k in range(NSUB):
                    rt = wk.tile([128, SUB], F32, tag="raw")
                    nc.scalar.activation(
                        rt[:], xs[jc][:, SUB * k: SUB * (k + 1)], AF.Tanh,
                        bias=b_sb[:, jc: jc + 1], scale=a_sb[:, jc: jc + 1],
                    )
                    nc.sync.dma_start(
                        rawT[128 * jc: 128 * (jc + 1), SUB * k: SUB * (k + 1)],
                        rt[:],
                    )

            # ---- main loop: outT = tanh(W'T.T @ x + b') ----
            for t in range(NT):
                for i in range(2):
                    ps = pm.tile([128, TW], F32, tag="pm")
                    for jc in range(2):
                        nc.tensor.matmul(
                            ps[:],
                            wpT_sb[:, D * jc + 128 * i: D * jc + 128 * (i + 1)],
                            xs[jc][:, TW * t: TW * (t + 1)],
                            start=(jc == 0), stop=(jc == 1),
                        )
                    ot = ob.tile([128, TW], F32, tag="o")
                    nc.scalar.activation(
                        ot[:], ps[:], AF.Tanh, bias=bp_sb[:, i: i + 1],
                    )
                    nc.sync.dma_start(
                        outT[128 * i: 128 * (i + 1), TW * t: TW * (t + 1)],
                        ot[:],
                    )

    nc.compile()
    _CACHE["nc"] = nc
    return nc


def make_in_maps(x, bn_gamma, bn_beta, W1, b1, W2, b2, Wf, bf):
    """Shard + lay out the full inputs for the 8 cores."""
    x = np.ascontiguousarray(np.asarray(x, dtype=np.float32))
    W1 = np.ascontiguousarray(np.asarray(W1, dtype=np.float32))
    W2 = np.ascontiguousarray(np.asarray(W2, dtype=np.float32))
    wfT = np.ascontiguousarray(np.asarray(Wf, dtype=np.float32).T)
    b1c = np.ascontiguousarray(np.asarray(b1, np.float32).reshape(4, 128).T)
    b2c = np.ascontiguousarray(np.asarray(b2, np.float32).reshape(16, 128).T)
    bfc = np.ascontiguousarray(np.asarray(bf, np.float32).reshape(2, 128).T)
    gamc = np.ascontiguousarray(np.asarray(bn_gamma, np.float32).reshape(2, 128).T)
    betc = np.ascontiguousarray(np.asarray(bn_beta, np.float32).reshape(2, 128).T)
    shared = dict(w1=W1, w2=W2, wfT=wfT, b1c=b1c, b2c=b2c, bfc=bfc,
                  gamc=gamc, betc=betc)
    in_maps = []
    for c in range(N_CORES):
        xs = np.ascontiguousarray(x[c * NR: (c + 1) * NR].T)
        in_maps.append(dict(xT=xs, **shared))
    return in_maps


def assemble(results):
    out = np.empty((N_FULL, D), np.float32)
    raw = np.empty((N_FULL, D), np.float32)
    for c in range(N_CORES):
        out[c * NR: (c + 1) * NR] = results[c]["outT"].T
        raw[c * NR: (c + 1) * NR] = results[c]["rawT"].T
    return out, raw


def kernel(x, bn_gamma, bn_beta, W1, b1, W2, b2, Wf, bf):
    nc = _build()
    in_maps = make_in_maps(x, bn_gamma, bn_beta, W1, b1, W2, b2, Wf, bf)
    res = bass_utils.run_bass_kernel_spmd(
        nc, in_maps, core_ids=list(range(N_CORES)))
    out, raw = assemble(res.results)
    return (out, raw)
